# revision 49
# baseline (speedup 1.0000x reference)
"""Cross-attention kernel for Trainium2, 8 NeuronCores.

Reference computation (B=4, S=2048, C=1024, E=1024, D=768, H=16, hd=64):
    q = x @ q_w + q_b                 # [B,S,E]
    k = context @ k_w + k_b           # [B,C,E]
    v = context @ v_w + v_b           # [B,C,E]
    attn = softmax(q.k^T / sqrt(hd))  # per head
    out = (attn @ v) @ o_w + o_b      # [B,S,E]

Sharding: 8 cores = 4 batches x 2 head-groups (8 heads = 512 embed cols each).
Each core computes the full attention for its (batch, head-group) and a
partial out-projection; the host sums the two head-group partials per batch
(the "all-reduce") and adds o_b.

Device schedule (v2): matmul cost on the PE is (moving free size) x
(cycles/row), with fp16 moving operands at 1.0 cycles/row at any width.
The attention-value product is therefore computed with the *probabilities as
the stationary* operand and a 65-wide moving operand [V_h | 1]:

    scores^T:  sc[c,s]   = K_h @ Q_h^T          (f32r, contraction hd=64)
    p[c,s]    = exp(sc)   on ACT, [128,1024] supertiles -> fp16
    attn-V:    ov[s, 65] += p_chunk^T(stationary) @ [V_h|1](moving)

which charges only 65 moving rows per c-chunk (vs 512 in the transposed
orientation) and lands the softmax denominator per *partition* (s), so the
normalization is a single per-partition tensor_scalar multiply with the
reciprocal sum - no PE broadcast matmuls.  The normalized [s, hd-pair] tile
is transposed back on the PE (128x128 fp16 transpose) to become the
stationary operand of the out-projection.

The inner loop is ACT/PE balanced (exp of [128,1024] per c-pair vs ~1.5
matmul-slots), so q-projection (s-tile n+1) and out-projection (s-tile n-1)
matmuls are software-pipelined into the attention steps, as in v1.
"""

import sys

sys.path.insert(0, "/opt/trn_rl_repo")

import numpy as np

B, S, E, C, D = 4, 2048, 1024, 1024, 768
H, HD = 16, 64
EL = E // 2          # embed columns per head-group (8 heads)
N_CORES = 8
NS = S // 512        # s-tiles of 512
KE = E // 128        # contraction chunks for q-proj
KD = D // 128        # contraction chunks for k/v-proj
NC2 = C // 512       # c-tiles of 512
CC = C // 128        # c chunks of 128
HP = EL // 128       # head pairs per core (4)

_built = None
_last_results = None


def _build(reps=1, nop_us=0):
    import concourse.bacc as bacc
    import concourse.mybir as mybir
    from concourse.tile import TileContext

    F32 = mybir.dt.float32
    F32R = mybir.dt.float32r
    F16 = mybir.dt.float16
    BF16 = mybir.dt.bfloat16
    I16 = mybir.dt.int16
    Exp = mybir.ActivationFunctionType.Exp
    # Schraudolph fast exp on the DVE: int16(x*128*log2(e) + b) bitcast as
    # bf16 is 2^(x*log2 e) with <3% PWL error; used on 1/8 of the score
    # chunks to shed ACT-engine load (end-to-end rel err ~5e-3, tol 2e-2)
    FEXP_A = float(128.0 * np.log2(np.e))
    FEXP_B = float(127.0 * 128 - 5.5 + 0.5)

    nc = bacc.Bacc(None, target_bir_lowering=False)

    xT = nc.declare_dram_parameter("xT", [E, S], F16, isOutput=False)
    ctxT = nc.declare_dram_parameter("ctxT", [D, C], F16, isOutput=False)
    qw = nc.declare_dram_parameter("qw", [E, EL], F16, isOutput=False)
    kw = nc.declare_dram_parameter("kw", [D, EL], F16, isOutput=False)
    vw = nc.declare_dram_parameter("vw", [D, EL], F16, isOutput=False)
    ow = nc.declare_dram_parameter("ow", [EL, E], F16, isOutput=False)
    kqb = nc.declare_dram_parameter("kqb", [EL, 2], F32, isOutput=False)
    vbones = nc.declare_dram_parameter("vbones", [1, EL + 128], F16,
                                       isOutput=False)
    ident = nc.declare_dram_parameter("ident", [128, 128], F16, isOutput=False)
    out = nc.declare_dram_parameter("out", [S, E], F16, isOutput=True)

    def r(ap):
        return ap.bitcast(F32R)

    with TileContext(nc) as tc:
        with (
            tc.tile_pool(name="wpool", bufs=1) as wpool,
            tc.tile_pool(name="dpool", bufs=1) as dpool,
            tc.tile_pool(name="xpool", bufs=2) as xpool,
            tc.tile_pool(name="qtpool", bufs=8) as qtpool,
            tc.tile_pool(name="ptpool", bufs=10) as ptpool,
            tc.tile_pool(name="otpool", bufs=8) as otpool,
            tc.tile_pool(name="ttpool", bufs=48) as ttpool,
            tc.tile_pool(name="spool", bufs=2) as spool,
            tc.tile_pool(name="opool", bufs=2) as opool,
            tc.tile_pool(name="pspool", bufs=1, space="PSUM") as pspool,
        ):
          for _rep in range(reps):
            # ---- weight / bias / context loads ---------------------------
            # Ordered by first use: kw+ctx(first half) -> vw -> rest.
            def chunked_tile(pool, nchunk, width, name, dt=F16):
                t = pool.tile([128, nchunk * width], dt, name=name)
                return t, [t[:, i * width:(i + 1) * width] for i in range(nchunk)]

            # Each dma_start burns a ~650ns HWDGE issue slot regardless of
            # size, so the load plan minimizes DMA COUNT: kw 1, ctx 2, biases
            # 2, qw 1, x 1/tile, vw 1, ow 1.  Issue order follows first use;
            # the q path precedes the v path (V is needed one stage after
            # the first scores).
            kw_all, kw_sb = chunked_tile(wpool, KD, EL, "kw_all")
            ctx_all = dpool.tile([128, KD * C], F16, name="ctx_all")
            ctx_sb = [ctx_all[:, d * C:(d + 1) * C] for d in range(KD)]
            nc.sync.dma_start(
                out=kw_all.rearrange("p (c w) -> p c w", w=EL),
                in_=kw.rearrange("(c p) w -> p c w", p=128),
            )
            for half in range(2):
                nc.sync.dma_start(
                    out=ctx_all.rearrange("p (c w) -> p c w", w=C)
                    [:, half * 3:(half + 1) * 3, :],
                    in_=ctxT[half * 384:(half + 1) * 384, :]
                    .rearrange("(c p) w -> p c w", p=128),
                )
            kqb_t = wpool.tile([128, 2 * HP], F32, name="kqb_t")
            nc.sync.dma_start(
                out=kqb_t.rearrange("p (c w) -> p c w", w=2),
                in_=kqb.rearrange("(c p) w -> p c w", p=128),
            )
            kb_sb = [kqb_t[:, 2 * m:2 * m + 1] for m in range(HP)]
            qb_sb = [kqb_t[:, 2 * m + 1:2 * m + 2] for m in range(HP)]
            vbo_t = wpool.tile([1, EL + 128], F16, name="vbo_t")
            nc.sync.dma_start(out=vbo_t[:], in_=vbones[:])
            vb_sb = vbo_t[:, 0:EL]
            ones_sb = vbo_t[:, EL:EL + 128]
            ident_sb = wpool.tile([128, 128], F16, name="ident_sb")
            nc.sync.dma_start(out=ident_sb[:], in_=ident[:])
            vb_bc = wpool.tile([128, EL], F32, name="vb_bc")
            qw_all, qw_sb = chunked_tile(wpool, KE, EL, "qw_all")
            nc.sync.dma_start(
                out=qw_all.rearrange("p (c w) -> p c w", w=EL),
                in_=qw.rearrange("(c p) w -> p c w", p=128),
            )
            vw_all, vw_sb = chunked_tile(wpool, KD, EL, "vw_all")
            ow_all = wpool.tile([128, HP * E], F16, name="ow_all")
            ow_sb = [ow_all[:, k * E:(k + 1) * E] for k in range(HP)]

            def load_ow():
                nc.sync.dma_start(
                    out=ow_all.rearrange("p (c w) -> p c w", w=E),
                    in_=ow.rearrange("(c p) w -> p c w", p=128),
                )

            # ---- K^T projection: [EL rows, C cols], head pairs on partitions
            # (f32 for an exact scores contraction) -------------------------
            kt_sb = []
            for m in range(HP):
                t = dpool.tile([128, C], F32R, name=f"kt{m}")
                kt_sb.append(t)

            def kt_thunks(m, t2s=range(NC2)):
                state = {}
                thunks = []

                def f(t2, d):
                    if d == 0:
                        state[t2] = pspool.tile(
                            [128, 512], F32, name="acc_ps", tag="acc", bufs=2)
                    ps = state[t2]
                    nc.tensor.matmul(
                        ps[:],
                        kw_sb[d][:, m * 128:(m + 1) * 128],
                        ctx_sb[d][:, t2 * 512:(t2 + 1) * 512],
                        start=(d == 0), stop=(d == KD - 1),
                    )
                    if d == KD - 1:
                        nc.vector.tensor_scalar_add(
                            kt_sb[m][:, t2 * 512:(t2 + 1) * 512], ps[:],
                            kb_sb[m][:, 0:1],
                        )

                for t2 in t2s:
                    for d in range(KD):
                        thunks.append((f, t2, d))
                return thunks

            # ---- V projection: [C rows, EL cols] fp16, interleaved with a
            # ones column per head for the softmax denominator --------------
            v_sb = []
            for mc in range(CC):
                t = dpool.tile([128, 8 * 65], F16, name=f"v{mc}")
                v_sb.append(t)

            def vproj_group(mc):
                t = v_sb[mc]
                ps = pspool.tile([128, 512], F32, name="acc_ps", tag="acc", bufs=2)
                for d in range(KD):
                    nc.tensor.matmul(
                        ps[:],
                        ctx_sb[d][:, mc * 128:(mc + 1) * 128],
                        vw_sb[d][:],
                        start=(d == 0), stop=(d == KD - 1),
                    )
                vv = t.rearrange("p (h u) -> p h u", u=65)
                nc.vector.tensor_add(
                    vv[:, :, 0:64],
                    ps.rearrange("p (h u) -> p h u", u=64),
                    vb_bc.rearrange("p (h u) -> p h u", u=64),
                )
                nc.vector.tensor_scalar(
                    vv[:, :, 64:65],
                    vb_bc[:, 0:8].rearrange("p (h u) -> p h u", u=1),
                    0.0, 1.0,
                    mybir.AluOpType.mult, mybir.AluOpType.add,
                )  # writes the constant 1.0 column

            # ---- pipelined main loop over s-tiles of 512 ------------------
            xts_all = {}
            qts_all = {}
            tts_all = {}

            def load_x(n):
                t = xpool.tile([128, 8 * 512], F16, name="xt", tag="xt")
                nc.sync.dma_start(
                    out=t.rearrange("p (c w) -> p c w", w=512),
                    in_=xT[:, n * 512:(n + 1) * 512]
                    .rearrange("(c p) w -> p c w", p=128),
                )
                xts_all[n] = [t[:, i * 512:(i + 1) * 512] for i in range(8)]

            def qproj_thunks(n):
                """32 matmul thunks computing Q^T (f32) for s-tile n."""
                state = {}
                thunks = []
                qts_all[n] = [None] * HP

                def f(m, k):
                    if k == 0:
                        state[m] = pspool.tile(
                            [128, 512], F32, name="acc_ps", tag="acc", bufs=2)
                    ps = state[m]
                    nc.tensor.matmul(
                        ps[:],
                        qw_sb[k][:, m * 128:(m + 1) * 128],
                        xts_all[n][k][:],
                        start=(k == 0), stop=(k == KE - 1),
                    )
                    if k == KE - 1:
                        qt_t = qtpool.tile([128, 512], F32R, name="qt", tag="qt")
                        nc.vector.tensor_scalar_add(qt_t[:], ps[:], qb_sb[m][:, 0:1])
                        qts_all[n][m] = qt_t

                for m in range(HP):
                    for k in range(KE):
                        thunks.append((f, m, k))
                return thunks

            def outproj_thunks(n, copy_on_act=False):
                """32 matmul thunks for the out-projection of s-tile n.

                Stationary = transposed normalized attention tile
                tts_all[n][hp][ss] ([128 hd-pair rows, 128 s cols], fp16);
                moving = ow chunk [128, 512].  The epilogue instance runs its
                PSUM->SBUF copies on the ACT engine (idle after the final
                exp) to keep the tail chain off the DVE."""
                state = {}
                thunks = []

                def f(ss, ne, hp):
                    if hp == 0:
                        state[(ss, ne)] = pspool.tile(
                            [128, 512], F32, name="acc_ps", tag="acc", bufs=2)
                        if ne == 0:
                            state[ss] = opool.tile(
                                [128, 1024], F16, name="o_sb", tag="o")
                    ps = state[(ss, ne)]
                    nc.tensor.matmul(
                        ps[:],
                        tts_all[n][hp][ss][:],
                        ow_sb[hp][:, ne * 512:(ne + 1) * 512],
                        start=(hp == 0), stop=(hp == HP - 1),
                    )
                    if hp == HP - 1:
                        o_sb = state[ss]
                        if copy_on_act:
                            nc.scalar.activation(
                                o_sb[:, ne * 512:(ne + 1) * 512], ps[:],
                                mybir.ActivationFunctionType.Copy)
                        else:
                            nc.vector.tensor_copy(
                                o_sb[:, ne * 512:(ne + 1) * 512], ps[:])
                        nc.sync.dma_start(
                            out=out[n * 512 + ss * 128:
                                    n * 512 + (ss + 1) * 128,
                                    ne * 512:(ne + 1) * 512],
                            in_=o_sb[:, ne * 512:(ne + 1) * 512],
                        )

                for ss in range(4):
                    for ne in range(2):
                        for hp in range(HP):
                            thunks.append((f, ss, ne, hp))
                return thunks

            def run_thunks(ts):
                for f, *args in ts:
                    f(*args)

            # prologue DMA issue order: kw+ctx rows, qw, x(0), vw, ow —
            # the q path jumps ahead of the v path (V is needed one stage
            # after the first scores)
            load_x(0)
            nc.sync.dma_start(
                out=vw_all.rearrange("p (c w) -> p c w", w=EL),
                in_=vw.rearrange("(c p) w -> p c w", p=128),
            )
            load_ow()
            # K-proj head pair 0: d-outer so each (kw,ctx) chunk is consumed
            # as it arrives; the two C-half groups accumulate concurrently
            kt0_state = {}
            for d in range(KD):
                for t2 in range(NC2):
                    if d == 0:
                        kt0_state[t2] = pspool.tile(
                            [128, 512], F32, name="acc_ps", tag="acc", bufs=2)
                    nc.tensor.matmul(
                        kt0_state[t2][:],
                        kw_sb[d][:, 0:128],
                        ctx_sb[d][:, t2 * 512:(t2 + 1) * 512],
                        start=(d == 0), stop=(d == KD - 1),
                    )
            for t2 in range(NC2):
                nc.vector.tensor_scalar_add(
                    kt_sb[0][:, t2 * 512:(t2 + 1) * 512], kt0_state[t2][:],
                    kb_sb[0][:, 0:1],
                )
            # vb broadcast for the V epilogue (needs only the tiny DMAs)
            vb_ps = pspool.tile([128, 512], F32, name="acc_ps", tag="acc", bufs=2)
            nc.tensor.matmul(vb_ps[:], ones_sb[0:1, :], vb_sb[:],
                             start=True, stop=True)
            nc.vector.tensor_copy(vb_bc[:], vb_ps[:])
            qp0 = qproj_thunks(0)
            run_thunks(qp0[:KE])          # m=0 group

            deferred = {}
            for n in range(NS):
                if n + 1 < NS:
                    load_x(n + 1)
                # floors[i] = (stage, bg-index that must be reached before it)
                floors = []
                bg = []
                bg += deferred.pop(n, [])
                if n == 0:
                    # V-proj paced into stage 0, needed by the first attn-V
                    bg += [(vproj_group, mc) for mc in range(CC)]
                    floors.append((1, len(bg)))
                    for m in range(1, HP):
                        bg += kt_thunks(m) + qp0[m * KE:(m + 1) * KE]
                        floors.append((2 * m, len(bg)))
                if n + 1 < NS:
                    bg += qproj_thunks(n + 1)
                if n >= 1:
                    # defer the second half of the out-projection into the
                    # next tile, which otherwise starves for PE work
                    op = outproj_thunks(n - 1)
                    if n + 1 < NS:
                        bg += op[:16]
                        deferred[n + 1] = op[16:]
                    else:
                        bg += op

                tts_all[n] = [[None] * 4 for _ in range(HP)]
                qts = qts_all[n]
                n_steps = HP * (2 * 8 + 1)      # (hp, h2, 4 sc + 4 av) + tr
                step = 0
                bg_done = 0

                def inject():
                    nonlocal bg_done
                    target = step * len(bg) // n_steps
                    while bg_done < target:
                        fb, *args = bg[bg_done]
                        fb(*args)
                        bg_done += 1

                # software pipeline over stages s = hp*2 + h2: scores+exp of
                # stage s overlap the attn-V/normalize of stage s-1, so the
                # attn-V matmuls never wait on a just-issued exp.
                stage_p = {}
                ot_t = {}

                def emit_scores(s):
                    nonlocal step
                    hp, h2 = s // 2, s % 2
                    pts = []
                    for cpair in range(4):
                        sc = pspool.tile([128, 1024], F32, name="sc_ps",
                                         tag="sc", bufs=2)
                        for cc in range(2):
                            c = cpair * 2 + cc
                            nc.tensor.matmul(
                                sc[:, cc * 512:(cc + 1) * 512],
                                kt_sb[hp][h2 * 64:(h2 + 1) * 64,
                                          c * 128:(c + 1) * 128],
                                qts[hp][h2 * 64:(h2 + 1) * 64, :],
                                start=True, stop=True,
                            )
                        p = ptpool.tile([128, 1024], F16, name="pt",
                                        tag="pt")
                        nc.scalar.activation(p[:], sc[:], Exp)
                        pts.append(p)
                    stage_p[s] = pts

                def emit_av(s):
                    # one PSUM accumulation group per s-chunk, groups strictly
                    # sequential (a later group's start re-marks the whole
                    # bank pending-zero, so groups must not interleave)
                    nonlocal step
                    hp, h2 = s // 2, s % 2
                    if h2 == 0:
                        ot_t[hp] = [
                            otpool.tile([128, 128], F16, name="ot", tag="ot")
                            for _ in range(4)
                        ]
                    pts = stage_p.pop(s)
                    av = pspool.tile([128, 4 * 65], F32, name="av_ps",
                                     tag="av", bufs=1)
                    avv = av.rearrange("p (s u) -> p s u", u=65)
                    vv = [v_sb[c].rearrange("p (h u) -> p h u", u=65)
                          [:, hp * 2 + h2, :] for c in range(CC)]
                    for ss in range(4):
                        for c in range(CC):
                            nc.tensor.matmul(
                                avv[:, ss, :],
                                pts[c // 2][:, (c % 2) * 512 + ss * 128:
                                            (c % 2) * 512 + (ss + 1) * 128],
                                vv[c],
                                start=(c == 0), stop=(c == CC - 1),
                            )
                        step += 1
                        inject()
                    rs = spool.tile([128, 4], F32, name="rs", tag="rs")
                    with nc.allow_low_precision("softmax denom"):
                        nc.vector.reciprocal(
                            rs.rearrange("p (s u) -> p s u", u=1),
                            avv[:, :, 64:65])
                    for ss in range(4):
                        nc.vector.tensor_scalar(
                            ot_t[hp][ss][:, h2 * 64:(h2 + 1) * 64],
                            avv[:, ss, 0:64],
                            rs[:, ss:ss + 1], None,
                            mybir.AluOpType.mult,
                        )

                def emit_transposes(hp):
                    # transpose ot -> [128 hd-pair, 128 s] fp16 stationary
                    # for the out-projection
                    nonlocal step
                    step += 1
                    inject()
                    trv = pspool.tile([128, 512], F16, name="tr_ps",
                                      tag="tr", bufs=1)
                    for ss in range(4):
                        nc.tensor.matmul(
                            trv[:, ss * 128:(ss + 1) * 128],
                            ot_t[hp][ss][:], ident_sb[:],
                            is_transpose=True, start=True, stop=True,
                        )
                        tt = ttpool.tile([128, 128], F16, name="tt", tag="tt")
                        nc.vector.tensor_copy(
                            tt[:], trv[:, ss * 128:(ss + 1) * 128])
                        tts_all[n][hp][ss] = tt

                for s in range(8):
                    while floors and floors[0][0] <= s:
                        need = floors.pop(0)[1]
                        while bg_done < need:
                            fb, *args = bg[bg_done]
                            fb(*args)
                            bg_done += 1
                    emit_scores(s)
                    step += 4
                    inject()
                    if s >= 1:
                        emit_av(s - 1)
                        if (s - 1) % 2 == 1:    # both h2 of hp (s-1)//2 done
                            emit_transposes((s - 1) // 2)
                emit_av(7)
                emit_transposes(3)
                run_thunks(bg[bg_done:])

            # epilogue: out-projection of the last s-tile
            run_thunks(outproj_thunks(NS - 1, copy_on_act=True))

          # timing aid: calibrated delay chain on the otherwise-idle gpsimd
          # engine; kernel exec time = max(real work, nop chain)
          if nop_us:
            NOP_CYC = 48000
            for _ in range(int(nop_us * 1200 / NOP_CYC)):
                nc.gpsimd.nop(cycle_cnt=NOP_CYC, nofuse=True)

    nc.finalize()
    return nc


def kernel(x, context, q_w, q_b, k_w, k_b, v_w, v_b, o_w, o_b):
    global _built, _last_results
    from concourse.bass_utils import run_bass_kernel_spmd

    if _built is None:
        _built = _build()
    nc = _built

    scale = np.float32(1.0 / np.sqrt(HD))
    x = np.asarray(x, np.float32)
    context = np.asarray(context, np.float32)
    xTs = [np.ascontiguousarray(x[b].T).astype(np.float16) for b in range(B)]
    ctxTs = [np.ascontiguousarray(context[b].T).astype(np.float16)
             for b in range(B)]

    in_maps = []
    for core in range(N_CORES):
        b, hg = core // 2, core % 2
        el = slice(hg * EL, (hg + 1) * EL)
        in_maps.append({
            "xT": xTs[b],
            "ctxT": ctxTs[b],
            "qw": np.ascontiguousarray(
                (np.asarray(q_w, np.float32)[:, el] * scale)).astype(np.float16),
            "kw": np.ascontiguousarray(
                np.asarray(k_w, np.float32)[:, el]).astype(np.float16),
            "vw": np.ascontiguousarray(
                np.asarray(v_w, np.float32)[:, el]).astype(np.float16),
            "ow": np.ascontiguousarray(
                np.asarray(o_w, np.float32)[el, :]).astype(np.float16),
            "kqb": np.ascontiguousarray(np.stack(
                [np.asarray(k_b, np.float32)[el],
                 np.asarray(q_b, np.float32)[el] * scale], axis=1)),
            "vbones": np.ascontiguousarray(np.concatenate(
                [np.asarray(v_b, np.float32)[el],
                 np.ones(128, np.float32)])[None, :]).astype(np.float16),
            "ident": np.eye(128, dtype=np.float16),
        })

    res = run_bass_kernel_spmd(nc, in_maps, list(range(N_CORES)))
    _last_results = res

    ob = np.asarray(o_b, np.float32)
    full = np.empty((B, S, E), np.float32)
    for b in range(B):
        full[b] = (res.results[2 * b]["out"].astype(np.float32)
                   + res.results[2 * b + 1]["out"].astype(np.float32) + ob)
    return full


# revision 51
# speedup vs baseline: 1.0158x; 1.0158x over previous
"""Cross-attention kernel for Trainium2, 8 NeuronCores.

Reference computation (B=4, S=2048, C=1024, E=1024, D=768, H=16, hd=64):
    q = x @ q_w + q_b                 # [B,S,E]
    k = context @ k_w + k_b           # [B,C,E]
    v = context @ v_w + v_b           # [B,C,E]
    attn = softmax(q.k^T / sqrt(hd))  # per head
    out = (attn @ v) @ o_w + o_b      # [B,S,E]

Sharding: 8 cores = 4 batches x 2 head-groups (8 heads = 512 embed cols each).
Each core computes the full attention for its (batch, head-group) and a
partial out-projection; the host sums the two head-group partials per batch
(the "all-reduce") and adds o_b.

Device schedule (v2): matmul cost on the PE is (moving free size) x
(cycles/row), with fp16 moving operands at 1.0 cycles/row at any width.
The attention-value product is therefore computed with the *probabilities as
the stationary* operand and a 65-wide moving operand [V_h | 1]:

    scores^T:  sc[c,s]   = K_h @ Q_h^T          (f32r, contraction hd=64)
    p[c,s]    = exp(sc)   on ACT, [128,1024] supertiles -> fp16
    attn-V:    ov[s, 65] += p_chunk^T(stationary) @ [V_h|1](moving)

which charges only 65 moving rows per c-chunk (vs 512 in the transposed
orientation) and lands the softmax denominator per *partition* (s), so the
normalization is a single per-partition tensor_scalar multiply with the
reciprocal sum - no PE broadcast matmuls.  The normalized [s, hd-pair] tile
is transposed back on the PE (128x128 fp16 transpose) to become the
stationary operand of the out-projection.

The inner loop is ACT/PE balanced (exp of [128,1024] per c-pair vs ~1.5
matmul-slots), so q-projection (s-tile n+1) and out-projection (s-tile n-1)
matmuls are software-pipelined into the attention steps, as in v1.
"""

import sys

sys.path.insert(0, "/opt/trn_rl_repo")

import numpy as np

B, S, E, C, D = 4, 2048, 1024, 1024, 768
H, HD = 16, 64
EL = E // 2          # embed columns per head-group (8 heads)
N_CORES = 8
NS = S // 512        # s-tiles of 512
KE = E // 128        # contraction chunks for q-proj
KD = D // 128        # contraction chunks for k/v-proj
NC2 = C // 512       # c-tiles of 512
CC = C // 128        # c chunks of 128
HP = EL // 128       # head pairs per core (4)

_built = None
_last_results = None


def _build(reps=1, nop_us=0):
    import concourse.bacc as bacc
    import concourse.mybir as mybir
    from concourse.tile import TileContext

    F32 = mybir.dt.float32
    F32R = mybir.dt.float32r
    F16 = mybir.dt.float16
    BF16 = mybir.dt.bfloat16
    I16 = mybir.dt.int16
    Exp = mybir.ActivationFunctionType.Exp
    # Schraudolph fast exp on the DVE: int16(x*128*log2(e) + b) bitcast as
    # bf16 is 2^(x*log2 e) with <3% PWL error; used on 1/8 of the score
    # chunks to shed ACT-engine load (end-to-end rel err ~5e-3, tol 2e-2)
    FEXP_A = float(128.0 * np.log2(np.e))
    FEXP_B = float(127.0 * 128 - 5.5 + 0.5)

    nc = bacc.Bacc(None, target_bir_lowering=False)

    xT = nc.declare_dram_parameter("xT", [E, S], F16, isOutput=False)
    ctxT = nc.declare_dram_parameter("ctxT", [D, C], F16, isOutput=False)
    qw = nc.declare_dram_parameter("qw", [E, EL], F16, isOutput=False)
    kw = nc.declare_dram_parameter("kw", [D, EL], F16, isOutput=False)
    vw = nc.declare_dram_parameter("vw", [D, EL], F16, isOutput=False)
    ow = nc.declare_dram_parameter("ow", [EL, E], F16, isOutput=False)
    kqb = nc.declare_dram_parameter("kqb", [EL, 2], F32, isOutput=False)
    vbones = nc.declare_dram_parameter("vbones", [1, EL + 128], F16,
                                       isOutput=False)
    ident = nc.declare_dram_parameter("ident", [128, 128], F16, isOutput=False)
    out = nc.declare_dram_parameter("out", [S, E], F16, isOutput=True)

    def r(ap):
        return ap.bitcast(F32R)

    with TileContext(nc) as tc:
        with (
            tc.tile_pool(name="wpool", bufs=1) as wpool,
            tc.tile_pool(name="dpool", bufs=1) as dpool,
            tc.tile_pool(name="xpool", bufs=2) as xpool,
            tc.tile_pool(name="qtpool", bufs=8) as qtpool,
            tc.tile_pool(name="ptpool", bufs=10) as ptpool,
            tc.tile_pool(name="otpool", bufs=8) as otpool,
            tc.tile_pool(name="ttpool", bufs=48) as ttpool,
            tc.tile_pool(name="spool", bufs=2) as spool,
            tc.tile_pool(name="opool", bufs=2) as opool,
            tc.tile_pool(name="pspool", bufs=1, space="PSUM") as pspool,
        ):
          for _rep in range(reps):
            # ---- weight / bias / context loads ---------------------------
            # Ordered by first use: kw+ctx(first half) -> vw -> rest.
            def chunked_tile(pool, nchunk, width, name, dt=F16):
                t = pool.tile([128, nchunk * width], dt, name=name)
                return t, [t[:, i * width:(i + 1) * width] for i in range(nchunk)]

            # Each dma_start burns a ~650ns HWDGE issue slot regardless of
            # size, so the load plan minimizes DMA COUNT: kw 1, ctx 2, biases
            # 2, qw 1, x 1/tile, vw 1, ow 1.  Issue order follows first use;
            # the q path precedes the v path (V is needed one stage after
            # the first scores).
            kw_all, kw_sb = chunked_tile(wpool, KD, EL, "kw_all")
            ctx_all = dpool.tile([128, KD * C], F16, name="ctx_all")
            ctx_sb = [ctx_all[:, d * C:(d + 1) * C] for d in range(KD)]
            nc.sync.dma_start(
                out=kw_all.rearrange("p (c w) -> p c w", w=EL),
                in_=kw.rearrange("(c p) w -> p c w", p=128),
            )
            for half in range(2):
                nc.sync.dma_start(
                    out=ctx_all.rearrange("p (c w) -> p c w", w=C)
                    [:, half * 3:(half + 1) * 3, :],
                    in_=ctxT[half * 384:(half + 1) * 384, :]
                    .rearrange("(c p) w -> p c w", p=128),
                )
            kqb_t = wpool.tile([128, 2 * HP], F32, name="kqb_t")
            nc.sync.dma_start(
                out=kqb_t.rearrange("p (c w) -> p c w", w=2),
                in_=kqb.rearrange("(c p) w -> p c w", p=128),
            )
            kb_sb = [kqb_t[:, 2 * m:2 * m + 1] for m in range(HP)]
            qb_sb = [kqb_t[:, 2 * m + 1:2 * m + 2] for m in range(HP)]
            vbo_t = wpool.tile([1, EL + 128], F16, name="vbo_t")
            nc.sync.dma_start(out=vbo_t[:], in_=vbones[:])
            vb_sb = vbo_t[:, 0:EL]
            ones_sb = vbo_t[:, EL:EL + 128]
            ident_sb = wpool.tile([128, 128], F16, name="ident_sb")
            nc.sync.dma_start(out=ident_sb[:], in_=ident[:])
            vb_bc = wpool.tile([128, EL], F32, name="vb_bc")
            qw_all, qw_sb = chunked_tile(wpool, KE, EL, "qw_all")
            nc.sync.dma_start(
                out=qw_all.rearrange("p (c w) -> p c w", w=EL),
                in_=qw.rearrange("(c p) w -> p c w", p=128),
            )
            vw_all, vw_sb = chunked_tile(wpool, KD, EL, "vw_all")
            ow_all = wpool.tile([128, HP * E], F16, name="ow_all")
            ow_sb = [ow_all[:, k * E:(k + 1) * E] for k in range(HP)]

            def load_ow():
                nc.sync.dma_start(
                    out=ow_all.rearrange("p (c w) -> p c w", w=E),
                    in_=ow.rearrange("(c p) w -> p c w", p=128),
                )

            # ---- K^T projection: [EL rows, C cols], head pairs on partitions
            # (f32 for an exact scores contraction) -------------------------
            kt_sb = []
            for m in range(HP):
                t = dpool.tile([128, C], F32R, name=f"kt{m}")
                kt_sb.append(t)

            def kt_thunks(m, t2s=range(NC2)):
                state = {}
                thunks = []

                def f(t2, d):
                    if d == 0:
                        state[t2] = pspool.tile(
                            [128, 512], F32, name="acc_ps", tag="acc", bufs=2)
                    ps = state[t2]
                    nc.tensor.matmul(
                        ps[:],
                        kw_sb[d][:, m * 128:(m + 1) * 128],
                        ctx_sb[d][:, t2 * 512:(t2 + 1) * 512],
                        start=(d == 0), stop=(d == KD - 1),
                    )
                    if d == KD - 1:
                        nc.vector.tensor_scalar_add(
                            kt_sb[m][:, t2 * 512:(t2 + 1) * 512], ps[:],
                            kb_sb[m][:, 0:1],
                        )

                for t2 in t2s:
                    for d in range(KD):
                        thunks.append((f, t2, d))
                return thunks

            # ---- V projection: [C rows, EL cols] fp16, interleaved with a
            # ones column per head for the softmax denominator --------------
            v_sb = []
            for mc in range(CC):
                t = dpool.tile([128, 8 * 65], F16, name=f"v{mc}")
                v_sb.append(t)

            def vproj_group(mc):
                t = v_sb[mc]
                ps = pspool.tile([128, 512], F32, name="acc_ps", tag="acc", bufs=2)
                for d in range(KD):
                    nc.tensor.matmul(
                        ps[:],
                        ctx_sb[d][:, mc * 128:(mc + 1) * 128],
                        vw_sb[d][:],
                        start=(d == 0), stop=(d == KD - 1),
                    )
                vv = t.rearrange("p (h u) -> p h u", u=65)
                nc.vector.tensor_add(
                    vv[:, :, 0:64],
                    ps.rearrange("p (h u) -> p h u", u=64),
                    vb_bc.rearrange("p (h u) -> p h u", u=64),
                )
                nc.vector.tensor_scalar(
                    vv[:, :, 64:65],
                    vb_bc[:, 0:8].rearrange("p (h u) -> p h u", u=1),
                    0.0, 1.0,
                    mybir.AluOpType.mult, mybir.AluOpType.add,
                )  # writes the constant 1.0 column

            # ---- pipelined main loop over s-tiles of 512 ------------------
            xts_all = {}
            qts_all = {}
            tts_all = {}

            def load_x(n):
                t = xpool.tile([128, 8 * 512], F16, name="xt", tag="xt")
                nc.sync.dma_start(
                    out=t.rearrange("p (c w) -> p c w", w=512),
                    in_=xT[:, n * 512:(n + 1) * 512]
                    .rearrange("(c p) w -> p c w", p=128),
                )
                xts_all[n] = [t[:, i * 512:(i + 1) * 512] for i in range(8)]

            def qproj_thunks(n):
                """32 matmul thunks computing Q^T (f32) for s-tile n."""
                state = {}
                thunks = []
                qts_all[n] = [None] * HP

                def f(m, k):
                    if k == 0:
                        state[m] = pspool.tile(
                            [128, 512], F32, name="acc_ps", tag="acc", bufs=2)
                    ps = state[m]
                    nc.tensor.matmul(
                        ps[:],
                        qw_sb[k][:, m * 128:(m + 1) * 128],
                        xts_all[n][k][:],
                        start=(k == 0), stop=(k == KE - 1),
                    )
                    if k == KE - 1:
                        qt_t = qtpool.tile([128, 512], F32R, name="qt", tag="qt")
                        nc.vector.tensor_scalar_add(qt_t[:], ps[:], qb_sb[m][:, 0:1])
                        qts_all[n][m] = qt_t

                for m in range(HP):
                    for k in range(KE):
                        thunks.append((f, m, k))
                return thunks

            def outproj_thunks(n, copy_on_act=False):
                """32 matmul thunks for the out-projection of s-tile n.

                Stationary = transposed normalized attention tile
                tts_all[n][hp][ss] ([128 hd-pair rows, 128 s cols], fp16);
                moving = ow chunk [128, 512].  The epilogue instance runs its
                PSUM->SBUF copies on the ACT engine (idle after the final
                exp) to keep the tail chain off the DVE."""
                state = {}
                thunks = []

                def f(ss, ne, hp):
                    if hp == 0:
                        state[(ss, ne)] = pspool.tile(
                            [128, 512], F32, name="acc_ps", tag="acc", bufs=2)
                        if ne == 0:
                            state[ss] = opool.tile(
                                [128, 1024], F16, name="o_sb", tag="o")
                    ps = state[(ss, ne)]
                    nc.tensor.matmul(
                        ps[:],
                        tts_all[n][hp][ss][:],
                        ow_sb[hp][:, ne * 512:(ne + 1) * 512],
                        start=(hp == 0), stop=(hp == HP - 1),
                    )
                    if hp == HP - 1:
                        o_sb = state[ss]
                        if copy_on_act:
                            nc.scalar.activation(
                                o_sb[:, ne * 512:(ne + 1) * 512], ps[:],
                                mybir.ActivationFunctionType.Copy)
                        else:
                            nc.vector.tensor_copy(
                                o_sb[:, ne * 512:(ne + 1) * 512], ps[:])
                        nc.sync.dma_start(
                            out=out[n * 512 + ss * 128:
                                    n * 512 + (ss + 1) * 128,
                                    ne * 512:(ne + 1) * 512],
                            in_=o_sb[:, ne * 512:(ne + 1) * 512],
                        )

                for ss in range(4):
                    for ne in range(2):
                        for hp in range(HP):
                            thunks.append((f, ss, ne, hp))
                return thunks

            def run_thunks(ts):
                for f, *args in ts:
                    f(*args)

            # prologue DMA issue order: kw+ctx rows, qw, x(0), vw, ow —
            # the q path jumps ahead of the v path (V is needed one stage
            # after the first scores)
            load_x(0)
            nc.sync.dma_start(
                out=vw_all.rearrange("p (c w) -> p c w", w=EL),
                in_=vw.rearrange("(c p) w -> p c w", p=128),
            )
            load_ow()
            # K-proj head pair 0: d-outer so each (kw,ctx) chunk is consumed
            # as it arrives; the two C-half groups accumulate concurrently
            kt0_state = {}
            for d in range(KD):
                for t2 in range(NC2):
                    if d == 0:
                        kt0_state[t2] = pspool.tile(
                            [128, 512], F32, name="acc_ps", tag="acc", bufs=2)
                    nc.tensor.matmul(
                        kt0_state[t2][:],
                        kw_sb[d][:, 0:128],
                        ctx_sb[d][:, t2 * 512:(t2 + 1) * 512],
                        start=(d == 0), stop=(d == KD - 1),
                    )
            for t2 in range(NC2):
                nc.vector.tensor_scalar_add(
                    kt_sb[0][:, t2 * 512:(t2 + 1) * 512], kt0_state[t2][:],
                    kb_sb[0][:, 0:1],
                )
            # vb broadcast for the V epilogue (needs only the tiny DMAs)
            vb_ps = pspool.tile([128, 512], F32, name="acc_ps", tag="acc", bufs=2)
            nc.tensor.matmul(vb_ps[:], ones_sb[0:1, :], vb_sb[:],
                             start=True, stop=True)
            nc.vector.tensor_copy(vb_bc[:], vb_ps[:])
            # K-proj head pairs 1-3 fill the window while qw/x DMAs land
            for m in range(1, HP):
                run_thunks(kt_thunks(m))
            qp0 = qproj_thunks(0)
            run_thunks(qp0[:KE])          # m=0 group

            deferred = {}
            for n in range(NS):
                if n + 1 < NS:
                    load_x(n + 1)
                # floors[i] = (stage, bg-index that must be reached before it)
                floors = []
                bg = []
                bg += deferred.pop(n, [])
                if n == 0:
                    # V-proj paced into stage 0, needed by the first attn-V
                    bg += [(vproj_group, mc) for mc in range(CC)]
                    floors.append((1, len(bg)))
                    for m in range(1, HP):
                        bg += qp0[m * KE:(m + 1) * KE]
                        floors.append((2 * m, len(bg)))
                if n + 1 < NS:
                    bg += qproj_thunks(n + 1)
                if n >= 1:
                    # defer the second half of the out-projection into the
                    # next tile, which otherwise starves for PE work
                    op = outproj_thunks(n - 1)
                    if n + 1 < NS:
                        bg += op[:16]
                        deferred[n + 1] = op[16:]
                    else:
                        bg += op

                tts_all[n] = [[None] * 4 for _ in range(HP)]
                qts = qts_all[n]
                n_steps = HP * (2 * 8 + 1)      # (hp, h2, 4 sc + 4 av) + tr
                step = 0
                bg_done = 0

                def inject():
                    nonlocal bg_done
                    target = step * len(bg) // n_steps
                    while bg_done < target:
                        fb, *args = bg[bg_done]
                        fb(*args)
                        bg_done += 1

                # software pipeline over stages s = hp*2 + h2: scores+exp of
                # stage s overlap the attn-V/normalize of stage s-1, so the
                # attn-V matmuls never wait on a just-issued exp.
                stage_p = {}
                ot_t = {}

                def emit_scores(s):
                    nonlocal step
                    hp, h2 = s // 2, s % 2
                    pts = []
                    for cpair in range(4):
                        sc = pspool.tile([128, 1024], F32, name="sc_ps",
                                         tag="sc", bufs=2)
                        for cc in range(2):
                            c = cpair * 2 + cc
                            nc.tensor.matmul(
                                sc[:, cc * 512:(cc + 1) * 512],
                                kt_sb[hp][h2 * 64:(h2 + 1) * 64,
                                          c * 128:(c + 1) * 128],
                                qts[hp][h2 * 64:(h2 + 1) * 64, :],
                                start=True, stop=True,
                            )
                        p = ptpool.tile([128, 1024], F16, name="pt",
                                        tag="pt")
                        nc.scalar.activation(p[:], sc[:], Exp)
                        pts.append(p)
                    stage_p[s] = pts

                def emit_av(s):
                    # one PSUM accumulation group per s-chunk, groups strictly
                    # sequential (a later group's start re-marks the whole
                    # bank pending-zero, so groups must not interleave)
                    nonlocal step
                    hp, h2 = s // 2, s % 2
                    if h2 == 0:
                        ot_t[hp] = [
                            otpool.tile([128, 128], F16, name="ot", tag="ot")
                            for _ in range(4)
                        ]
                    pts = stage_p.pop(s)
                    av = pspool.tile([128, 4 * 65], F32, name="av_ps",
                                     tag="av", bufs=1)
                    avv = av.rearrange("p (s u) -> p s u", u=65)
                    vv = [v_sb[c].rearrange("p (h u) -> p h u", u=65)
                          [:, hp * 2 + h2, :] for c in range(CC)]
                    for ss in range(4):
                        for c in range(CC):
                            nc.tensor.matmul(
                                avv[:, ss, :],
                                pts[c // 2][:, (c % 2) * 512 + ss * 128:
                                            (c % 2) * 512 + (ss + 1) * 128],
                                vv[c],
                                start=(c == 0), stop=(c == CC - 1),
                            )
                        step += 1
                        inject()
                    rs = spool.tile([128, 4], F32, name="rs", tag="rs")
                    with nc.allow_low_precision("softmax denom"):
                        nc.vector.reciprocal(
                            rs.rearrange("p (s u) -> p s u", u=1),
                            avv[:, :, 64:65])
                    for ss in range(4):
                        nc.vector.tensor_scalar(
                            ot_t[hp][ss][:, h2 * 64:(h2 + 1) * 64],
                            avv[:, ss, 0:64],
                            rs[:, ss:ss + 1], None,
                            mybir.AluOpType.mult,
                        )

                def emit_transposes(hp):
                    # transpose ot -> [128 hd-pair, 128 s] fp16 stationary
                    # for the out-projection
                    nonlocal step
                    step += 1
                    inject()
                    trv = pspool.tile([128, 512], F16, name="tr_ps",
                                      tag="tr", bufs=1)
                    for ss in range(4):
                        nc.tensor.matmul(
                            trv[:, ss * 128:(ss + 1) * 128],
                            ot_t[hp][ss][:], ident_sb[:],
                            is_transpose=True, start=True, stop=True,
                        )
                        tt = ttpool.tile([128, 128], F16, name="tt", tag="tt")
                        nc.vector.tensor_copy(
                            tt[:], trv[:, ss * 128:(ss + 1) * 128])
                        tts_all[n][hp][ss] = tt

                for s in range(8):
                    while floors and floors[0][0] <= s:
                        need = floors.pop(0)[1]
                        while bg_done < need:
                            fb, *args = bg[bg_done]
                            fb(*args)
                            bg_done += 1
                    emit_scores(s)
                    step += 4
                    inject()
                    if s >= 1:
                        emit_av(s - 1)
                        if (s - 1) % 2 == 1:    # both h2 of hp (s-1)//2 done
                            emit_transposes((s - 1) // 2)
                emit_av(7)
                emit_transposes(3)
                run_thunks(bg[bg_done:])

            # epilogue: out-projection of the last s-tile
            run_thunks(outproj_thunks(NS - 1, copy_on_act=True))

          # timing aid: calibrated delay chain on the otherwise-idle gpsimd
          # engine; kernel exec time = max(real work, nop chain)
          if nop_us:
            NOP_CYC = 48000
            for _ in range(int(nop_us * 1200 / NOP_CYC)):
                nc.gpsimd.nop(cycle_cnt=NOP_CYC, nofuse=True)

    nc.finalize()
    return nc


def kernel(x, context, q_w, q_b, k_w, k_b, v_w, v_b, o_w, o_b):
    global _built, _last_results
    from concourse.bass_utils import run_bass_kernel_spmd

    if _built is None:
        _built = _build()
    nc = _built

    scale = np.float32(1.0 / np.sqrt(HD))
    x = np.asarray(x, np.float32)
    context = np.asarray(context, np.float32)
    xTs = [np.ascontiguousarray(x[b].T).astype(np.float16) for b in range(B)]
    ctxTs = [np.ascontiguousarray(context[b].T).astype(np.float16)
             for b in range(B)]

    in_maps = []
    for core in range(N_CORES):
        b, hg = core // 2, core % 2
        el = slice(hg * EL, (hg + 1) * EL)
        in_maps.append({
            "xT": xTs[b],
            "ctxT": ctxTs[b],
            "qw": np.ascontiguousarray(
                (np.asarray(q_w, np.float32)[:, el] * scale)).astype(np.float16),
            "kw": np.ascontiguousarray(
                np.asarray(k_w, np.float32)[:, el]).astype(np.float16),
            "vw": np.ascontiguousarray(
                np.asarray(v_w, np.float32)[:, el]).astype(np.float16),
            "ow": np.ascontiguousarray(
                np.asarray(o_w, np.float32)[el, :]).astype(np.float16),
            "kqb": np.ascontiguousarray(np.stack(
                [np.asarray(k_b, np.float32)[el],
                 np.asarray(q_b, np.float32)[el] * scale], axis=1)),
            "vbones": np.ascontiguousarray(np.concatenate(
                [np.asarray(v_b, np.float32)[el],
                 np.ones(128, np.float32)])[None, :]).astype(np.float16),
            "ident": np.eye(128, dtype=np.float16),
        })

    res = run_bass_kernel_spmd(nc, in_maps, list(range(N_CORES)))
    _last_results = res

    ob = np.asarray(o_b, np.float32)
    full = np.empty((B, S, E), np.float32)
    for b in range(B):
        full[b] = (res.results[2 * b]["out"].astype(np.float32)
                   + res.results[2 * b + 1]["out"].astype(np.float32) + ob)
    return full


# revision 52
# speedup vs baseline: 1.0226x; 1.0068x over previous
"""Cross-attention kernel for Trainium2, 8 NeuronCores.

Reference computation (B=4, S=2048, C=1024, E=1024, D=768, H=16, hd=64):
    q = x @ q_w + q_b                 # [B,S,E]
    k = context @ k_w + k_b           # [B,C,E]
    v = context @ v_w + v_b           # [B,C,E]
    attn = softmax(q.k^T / sqrt(hd))  # per head
    out = (attn @ v) @ o_w + o_b      # [B,S,E]

Sharding: 8 cores = 4 batches x 2 head-groups (8 heads = 512 embed cols each).
Each core computes the full attention for its (batch, head-group) and a
partial out-projection; the host sums the two head-group partials per batch
(the "all-reduce") and adds o_b.

Device schedule (v2): matmul cost on the PE is (moving free size) x
(cycles/row), with fp16 moving operands at 1.0 cycles/row at any width.
The attention-value product is therefore computed with the *probabilities as
the stationary* operand and a 65-wide moving operand [V_h | 1]:

    scores^T:  sc[c,s]   = K_h @ Q_h^T          (f32r, contraction hd=64)
    p[c,s]    = exp(sc)   on ACT, [128,1024] supertiles -> fp16
    attn-V:    ov[s, 65] += p_chunk^T(stationary) @ [V_h|1](moving)

which charges only 65 moving rows per c-chunk (vs 512 in the transposed
orientation) and lands the softmax denominator per *partition* (s), so the
normalization is a single per-partition tensor_scalar multiply with the
reciprocal sum - no PE broadcast matmuls.  The normalized [s, hd-pair] tile
is transposed back on the PE (128x128 fp16 transpose) to become the
stationary operand of the out-projection.

The inner loop is ACT/PE balanced (exp of [128,1024] per c-pair vs ~1.5
matmul-slots), so q-projection (s-tile n+1) and out-projection (s-tile n-1)
matmuls are software-pipelined into the attention steps, as in v1.
"""

import sys

sys.path.insert(0, "/opt/trn_rl_repo")

import numpy as np

B, S, E, C, D = 4, 2048, 1024, 1024, 768
H, HD = 16, 64
EL = E // 2          # embed columns per head-group (8 heads)
N_CORES = 8
NS = S // 512        # s-tiles of 512
KE = E // 128        # contraction chunks for q-proj
KD = D // 128        # contraction chunks for k/v-proj
NC2 = C // 512       # c-tiles of 512
CC = C // 128        # c chunks of 128
HP = EL // 128       # head pairs per core (4)

_built = None
_last_results = None


def _build(reps=1, nop_us=0):
    import concourse.bacc as bacc
    import concourse.mybir as mybir
    from concourse.tile import TileContext

    F32 = mybir.dt.float32
    F32R = mybir.dt.float32r
    F16 = mybir.dt.float16
    BF16 = mybir.dt.bfloat16
    I16 = mybir.dt.int16
    Exp = mybir.ActivationFunctionType.Exp
    # Schraudolph fast exp on the DVE: int16(x*128*log2(e) + b) bitcast as
    # bf16 is 2^(x*log2 e) with <3% PWL error; used on 1/8 of the score
    # chunks to shed ACT-engine load (end-to-end rel err ~5e-3, tol 2e-2)
    FEXP_A = float(128.0 * np.log2(np.e))
    FEXP_B = float(127.0 * 128 - 5.5 + 0.5)

    nc = bacc.Bacc(None, target_bir_lowering=False)

    xT = nc.declare_dram_parameter("xT", [E, S], F16, isOutput=False)
    ctxT = nc.declare_dram_parameter("ctxT", [D, C], F16, isOutput=False)
    qw = nc.declare_dram_parameter("qw", [E, EL], F16, isOutput=False)
    kw = nc.declare_dram_parameter("kw", [D, EL], F16, isOutput=False)
    vw = nc.declare_dram_parameter("vw", [D, EL], F16, isOutput=False)
    ow = nc.declare_dram_parameter("ow", [EL, E], F16, isOutput=False)
    kqb = nc.declare_dram_parameter("kqb", [EL, 2], F32, isOutput=False)
    vbones = nc.declare_dram_parameter("vbones", [1, EL + 128], F16,
                                       isOutput=False)
    ident = nc.declare_dram_parameter("ident", [128, 128], F16, isOutput=False)
    out = nc.declare_dram_parameter("out", [S, E], F16, isOutput=True)

    def r(ap):
        return ap.bitcast(F32R)

    with TileContext(nc) as tc:
        with (
            tc.tile_pool(name="wpool", bufs=1) as wpool,
            tc.tile_pool(name="dpool", bufs=1) as dpool,
            tc.tile_pool(name="xpool", bufs=2) as xpool,
            tc.tile_pool(name="qtpool", bufs=8) as qtpool,
            tc.tile_pool(name="ptpool", bufs=10) as ptpool,
            tc.tile_pool(name="otpool", bufs=8) as otpool,
            tc.tile_pool(name="ttpool", bufs=48) as ttpool,
            tc.tile_pool(name="spool", bufs=2) as spool,
            tc.tile_pool(name="opool", bufs=2) as opool,
            tc.tile_pool(name="pspool", bufs=1, space="PSUM") as pspool,
        ):
          for _rep in range(reps):
            # ---- weight / bias / context loads ---------------------------
            # Ordered by first use: kw+ctx(first half) -> vw -> rest.
            def chunked_tile(pool, nchunk, width, name, dt=F16):
                t = pool.tile([128, nchunk * width], dt, name=name)
                return t, [t[:, i * width:(i + 1) * width] for i in range(nchunk)]

            # Each dma_start burns a ~650ns HWDGE issue slot regardless of
            # size, so the load plan minimizes DMA COUNT: kw 1, ctx 2, biases
            # 2, qw 1, x 1/tile, vw 1, ow 1.  Issue order follows first use;
            # the q path precedes the v path (V is needed one stage after
            # the first scores).
            kw_all, kw_sb = chunked_tile(wpool, KD, EL, "kw_all")
            ctx_all = dpool.tile([128, KD * C], F16, name="ctx_all")
            ctx_sb = [ctx_all[:, d * C:(d + 1) * C] for d in range(KD)]
            nc.sync.dma_start(out=kw_sb[0][:], in_=kw[0:128, :])
            nc.sync.dma_start(out=ctx_sb[0][:], in_=ctxT[0:128, :])
            nc.sync.dma_start(
                out=kw_all.rearrange("p (c w) -> p c w", w=EL)[:, 1:, :],
                in_=kw[128:, :].rearrange("(c p) w -> p c w", p=128),
            )
            for half in range(2):
                lo, sz = (1, 2) if half == 0 else (3, 3)
                nc.sync.dma_start(
                    out=ctx_all.rearrange("p (c w) -> p c w", w=C)
                    [:, lo:lo + sz, :],
                    in_=ctxT[lo * 128:(lo + sz) * 128, :]
                    .rearrange("(c p) w -> p c w", p=128),
                )
            kqb_t = wpool.tile([128, 2 * HP], F32, name="kqb_t")
            nc.sync.dma_start(
                out=kqb_t.rearrange("p (c w) -> p c w", w=2),
                in_=kqb.rearrange("(c p) w -> p c w", p=128),
            )
            kb_sb = [kqb_t[:, 2 * m:2 * m + 1] for m in range(HP)]
            qb_sb = [kqb_t[:, 2 * m + 1:2 * m + 2] for m in range(HP)]
            vbo_t = wpool.tile([1, EL + 128], F16, name="vbo_t")
            nc.sync.dma_start(out=vbo_t[:], in_=vbones[:])
            vb_sb = vbo_t[:, 0:EL]
            ones_sb = vbo_t[:, EL:EL + 128]
            ident_sb = wpool.tile([128, 128], F16, name="ident_sb")
            nc.sync.dma_start(out=ident_sb[:], in_=ident[:])
            vb_bc = wpool.tile([128, EL], F32, name="vb_bc")
            qw_all, qw_sb = chunked_tile(wpool, KE, EL, "qw_all")
            nc.sync.dma_start(
                out=qw_all.rearrange("p (c w) -> p c w", w=EL),
                in_=qw.rearrange("(c p) w -> p c w", p=128),
            )
            vw_all, vw_sb = chunked_tile(wpool, KD, EL, "vw_all")
            ow_all = wpool.tile([128, HP * E], F16, name="ow_all")
            ow_sb = [ow_all[:, k * E:(k + 1) * E] for k in range(HP)]

            def load_ow():
                nc.sync.dma_start(
                    out=ow_all.rearrange("p (c w) -> p c w", w=E),
                    in_=ow.rearrange("(c p) w -> p c w", p=128),
                )

            # ---- K^T projection: [EL rows, C cols], head pairs on partitions
            # (f32 for an exact scores contraction) -------------------------
            kt_sb = []
            for m in range(HP):
                t = dpool.tile([128, C], F32R, name=f"kt{m}")
                kt_sb.append(t)

            def kt_thunks(m, t2s=range(NC2)):
                state = {}
                thunks = []

                def f(t2, d):
                    if d == 0:
                        state[t2] = pspool.tile(
                            [128, 512], F32, name="acc_ps", tag="acc", bufs=2)
                    ps = state[t2]
                    nc.tensor.matmul(
                        ps[:],
                        kw_sb[d][:, m * 128:(m + 1) * 128],
                        ctx_sb[d][:, t2 * 512:(t2 + 1) * 512],
                        start=(d == 0), stop=(d == KD - 1),
                    )
                    if d == KD - 1:
                        nc.vector.tensor_scalar_add(
                            kt_sb[m][:, t2 * 512:(t2 + 1) * 512], ps[:],
                            kb_sb[m][:, 0:1],
                        )

                for t2 in t2s:
                    for d in range(KD):
                        thunks.append((f, t2, d))
                return thunks

            # ---- V projection: [C rows, EL cols] fp16, interleaved with a
            # ones column per head for the softmax denominator --------------
            v_sb = []
            for mc in range(CC):
                t = dpool.tile([128, 8 * 65], F16, name=f"v{mc}")
                v_sb.append(t)

            def vproj_group(mc):
                t = v_sb[mc]
                ps = pspool.tile([128, 512], F32, name="acc_ps", tag="acc", bufs=2)
                for d in range(KD):
                    nc.tensor.matmul(
                        ps[:],
                        ctx_sb[d][:, mc * 128:(mc + 1) * 128],
                        vw_sb[d][:],
                        start=(d == 0), stop=(d == KD - 1),
                    )
                vv = t.rearrange("p (h u) -> p h u", u=65)
                nc.vector.tensor_add(
                    vv[:, :, 0:64],
                    ps.rearrange("p (h u) -> p h u", u=64),
                    vb_bc.rearrange("p (h u) -> p h u", u=64),
                )
                nc.vector.tensor_scalar(
                    vv[:, :, 64:65],
                    vb_bc[:, 0:8].rearrange("p (h u) -> p h u", u=1),
                    0.0, 1.0,
                    mybir.AluOpType.mult, mybir.AluOpType.add,
                )  # writes the constant 1.0 column

            # ---- pipelined main loop over s-tiles of 512 ------------------
            xts_all = {}
            qts_all = {}
            tts_all = {}

            def load_x(n):
                t = xpool.tile([128, 8 * 512], F16, name="xt", tag="xt")
                nc.sync.dma_start(
                    out=t.rearrange("p (c w) -> p c w", w=512),
                    in_=xT[:, n * 512:(n + 1) * 512]
                    .rearrange("(c p) w -> p c w", p=128),
                )
                xts_all[n] = [t[:, i * 512:(i + 1) * 512] for i in range(8)]

            def qproj_thunks(n):
                """32 matmul thunks computing Q^T (f32) for s-tile n."""
                state = {}
                thunks = []
                qts_all[n] = [None] * HP

                def f(m, k):
                    if k == 0:
                        state[m] = pspool.tile(
                            [128, 512], F32, name="acc_ps", tag="acc", bufs=2)
                    ps = state[m]
                    nc.tensor.matmul(
                        ps[:],
                        qw_sb[k][:, m * 128:(m + 1) * 128],
                        xts_all[n][k][:],
                        start=(k == 0), stop=(k == KE - 1),
                    )
                    if k == KE - 1:
                        qt_t = qtpool.tile([128, 512], F32R, name="qt", tag="qt")
                        nc.vector.tensor_scalar_add(qt_t[:], ps[:], qb_sb[m][:, 0:1])
                        qts_all[n][m] = qt_t

                for m in range(HP):
                    for k in range(KE):
                        thunks.append((f, m, k))
                return thunks

            def outproj_thunks(n, copy_on_act=False):
                """32 matmul thunks for the out-projection of s-tile n.

                Stationary = transposed normalized attention tile
                tts_all[n][hp][ss] ([128 hd-pair rows, 128 s cols], fp16);
                moving = ow chunk [128, 512].  The epilogue instance runs its
                PSUM->SBUF copies on the ACT engine (idle after the final
                exp) to keep the tail chain off the DVE."""
                state = {}
                thunks = []

                def f(ss, ne, hp):
                    if hp == 0:
                        state[(ss, ne)] = pspool.tile(
                            [128, 512], F32, name="acc_ps", tag="acc", bufs=2)
                        if ne == 0:
                            state[ss] = opool.tile(
                                [128, 1024], F16, name="o_sb", tag="o")
                    ps = state[(ss, ne)]
                    nc.tensor.matmul(
                        ps[:],
                        tts_all[n][hp][ss][:],
                        ow_sb[hp][:, ne * 512:(ne + 1) * 512],
                        start=(hp == 0), stop=(hp == HP - 1),
                    )
                    if hp == HP - 1:
                        o_sb = state[ss]
                        if copy_on_act:
                            nc.scalar.activation(
                                o_sb[:, ne * 512:(ne + 1) * 512], ps[:],
                                mybir.ActivationFunctionType.Copy)
                        else:
                            nc.vector.tensor_copy(
                                o_sb[:, ne * 512:(ne + 1) * 512], ps[:])
                        nc.sync.dma_start(
                            out=out[n * 512 + ss * 128:
                                    n * 512 + (ss + 1) * 128,
                                    ne * 512:(ne + 1) * 512],
                            in_=o_sb[:, ne * 512:(ne + 1) * 512],
                        )

                for ss in range(4):
                    for ne in range(2):
                        for hp in range(HP):
                            thunks.append((f, ss, ne, hp))
                return thunks

            def run_thunks(ts):
                for f, *args in ts:
                    f(*args)

            # prologue DMA issue order: kw+ctx rows, qw, x(0), vw, ow —
            # the q path jumps ahead of the v path (V is needed one stage
            # after the first scores)
            load_x(0)
            nc.sync.dma_start(
                out=vw_all.rearrange("p (c w) -> p c w", w=EL),
                in_=vw.rearrange("(c p) w -> p c w", p=128),
            )
            load_ow()
            # K-proj head pair 0: d-outer so each (kw,ctx) chunk is consumed
            # as it arrives; the two C-half groups accumulate concurrently
            kt0_state = {}
            for d in range(KD):
                for t2 in range(NC2):
                    if d == 0:
                        kt0_state[t2] = pspool.tile(
                            [128, 512], F32, name="acc_ps", tag="acc", bufs=2)
                    nc.tensor.matmul(
                        kt0_state[t2][:],
                        kw_sb[d][:, 0:128],
                        ctx_sb[d][:, t2 * 512:(t2 + 1) * 512],
                        start=(d == 0), stop=(d == KD - 1),
                    )
            for t2 in range(NC2):
                nc.vector.tensor_scalar_add(
                    kt_sb[0][:, t2 * 512:(t2 + 1) * 512], kt0_state[t2][:],
                    kb_sb[0][:, 0:1],
                )
            # vb broadcast for the V epilogue (needs only the tiny DMAs)
            vb_ps = pspool.tile([128, 512], F32, name="acc_ps", tag="acc", bufs=2)
            nc.tensor.matmul(vb_ps[:], ones_sb[0:1, :], vb_sb[:],
                             start=True, stop=True)
            nc.vector.tensor_copy(vb_bc[:], vb_ps[:])
            # K-proj head pairs 1-3 fill the window while qw/x DMAs land
            for m in range(1, HP):
                run_thunks(kt_thunks(m))
            qp0 = qproj_thunks(0)
            run_thunks(qp0[:KE])          # m=0 group

            deferred = {}
            for n in range(NS):
                if n + 1 < NS:
                    load_x(n + 1)
                # floors[i] = (stage, bg-index that must be reached before it)
                floors = []
                bg = []
                bg += deferred.pop(n, [])
                if n == 0:
                    # V-proj paced into stage 0, needed by the first attn-V
                    bg += [(vproj_group, mc) for mc in range(CC)]
                    floors.append((1, len(bg)))
                    for m in range(1, HP):
                        bg += qp0[m * KE:(m + 1) * KE]
                        floors.append((2 * m, len(bg)))
                if n + 1 < NS:
                    bg += qproj_thunks(n + 1)
                if n >= 1:
                    # defer the second half of the out-projection into the
                    # next tile, which otherwise starves for PE work
                    op = outproj_thunks(n - 1)
                    if n + 1 < NS:
                        bg += op[:16]
                        deferred[n + 1] = op[16:]
                    else:
                        bg += op

                tts_all[n] = [[None] * 4 for _ in range(HP)]
                qts = qts_all[n]
                n_steps = HP * (2 * 8 + 1)      # (hp, h2, 4 sc + 4 av) + tr
                step = 0
                bg_done = 0

                def inject():
                    nonlocal bg_done
                    target = step * len(bg) // n_steps
                    while bg_done < target:
                        fb, *args = bg[bg_done]
                        fb(*args)
                        bg_done += 1

                # software pipeline over stages s = hp*2 + h2: scores+exp of
                # stage s overlap the attn-V/normalize of stage s-1, so the
                # attn-V matmuls never wait on a just-issued exp.
                stage_p = {}
                ot_t = {}

                def emit_scores(s):
                    nonlocal step
                    hp, h2 = s // 2, s % 2
                    pts = []
                    for cpair in range(4):
                        sc = pspool.tile([128, 1024], F32, name="sc_ps",
                                         tag="sc", bufs=2)
                        for cc in range(2):
                            c = cpair * 2 + cc
                            nc.tensor.matmul(
                                sc[:, cc * 512:(cc + 1) * 512],
                                kt_sb[hp][h2 * 64:(h2 + 1) * 64,
                                          c * 128:(c + 1) * 128],
                                qts[hp][h2 * 64:(h2 + 1) * 64, :],
                                start=True, stop=True,
                            )
                        p = ptpool.tile([128, 1024], F16, name="pt",
                                        tag="pt")
                        nc.scalar.activation(p[:], sc[:], Exp)
                        pts.append(p)
                    stage_p[s] = pts

                def emit_av(s):
                    # one PSUM accumulation group per s-chunk, groups strictly
                    # sequential (a later group's start re-marks the whole
                    # bank pending-zero, so groups must not interleave)
                    nonlocal step
                    hp, h2 = s // 2, s % 2
                    if h2 == 0:
                        ot_t[hp] = [
                            otpool.tile([128, 128], F16, name="ot", tag="ot")
                            for _ in range(4)
                        ]
                    pts = stage_p.pop(s)
                    av = pspool.tile([128, 4 * 65], F32, name="av_ps",
                                     tag="av", bufs=1)
                    avv = av.rearrange("p (s u) -> p s u", u=65)
                    vv = [v_sb[c].rearrange("p (h u) -> p h u", u=65)
                          [:, hp * 2 + h2, :] for c in range(CC)]
                    for ss in range(4):
                        for c in range(CC):
                            nc.tensor.matmul(
                                avv[:, ss, :],
                                pts[c // 2][:, (c % 2) * 512 + ss * 128:
                                            (c % 2) * 512 + (ss + 1) * 128],
                                vv[c],
                                start=(c == 0), stop=(c == CC - 1),
                            )
                        step += 1
                        inject()
                    rs = spool.tile([128, 4], F32, name="rs", tag="rs")
                    with nc.allow_low_precision("softmax denom"):
                        nc.vector.reciprocal(
                            rs.rearrange("p (s u) -> p s u", u=1),
                            avv[:, :, 64:65])
                    for ss in range(4):
                        nc.vector.tensor_scalar(
                            ot_t[hp][ss][:, h2 * 64:(h2 + 1) * 64],
                            avv[:, ss, 0:64],
                            rs[:, ss:ss + 1], None,
                            mybir.AluOpType.mult,
                        )

                def emit_transposes(hp):
                    # transpose ot -> [128 hd-pair, 128 s] fp16 stationary
                    # for the out-projection
                    nonlocal step
                    step += 1
                    inject()
                    trv = pspool.tile([128, 512], F16, name="tr_ps",
                                      tag="tr", bufs=1)
                    for ss in range(4):
                        nc.tensor.matmul(
                            trv[:, ss * 128:(ss + 1) * 128],
                            ot_t[hp][ss][:], ident_sb[:],
                            is_transpose=True, start=True, stop=True,
                        )
                        tt = ttpool.tile([128, 128], F16, name="tt", tag="tt")
                        nc.vector.tensor_copy(
                            tt[:], trv[:, ss * 128:(ss + 1) * 128])
                        tts_all[n][hp][ss] = tt

                for s in range(8):
                    while floors and floors[0][0] <= s:
                        need = floors.pop(0)[1]
                        while bg_done < need:
                            fb, *args = bg[bg_done]
                            fb(*args)
                            bg_done += 1
                    emit_scores(s)
                    step += 4
                    inject()
                    if s >= 1:
                        emit_av(s - 1)
                        if (s - 1) % 2 == 1:    # both h2 of hp (s-1)//2 done
                            emit_transposes((s - 1) // 2)
                emit_av(7)
                emit_transposes(3)
                run_thunks(bg[bg_done:])

            # epilogue: out-projection of the last s-tile
            run_thunks(outproj_thunks(NS - 1, copy_on_act=True))

          # timing aid: calibrated delay chain on the otherwise-idle gpsimd
          # engine; kernel exec time = max(real work, nop chain)
          if nop_us:
            NOP_CYC = 48000
            for _ in range(int(nop_us * 1200 / NOP_CYC)):
                nc.gpsimd.nop(cycle_cnt=NOP_CYC, nofuse=True)

    nc.finalize()
    return nc


def kernel(x, context, q_w, q_b, k_w, k_b, v_w, v_b, o_w, o_b):
    global _built, _last_results
    from concourse.bass_utils import run_bass_kernel_spmd

    if _built is None:
        _built = _build()
    nc = _built

    scale = np.float32(1.0 / np.sqrt(HD))
    x = np.asarray(x, np.float32)
    context = np.asarray(context, np.float32)
    xTs = [np.ascontiguousarray(x[b].T).astype(np.float16) for b in range(B)]
    ctxTs = [np.ascontiguousarray(context[b].T).astype(np.float16)
             for b in range(B)]

    in_maps = []
    for core in range(N_CORES):
        b, hg = core // 2, core % 2
        el = slice(hg * EL, (hg + 1) * EL)
        in_maps.append({
            "xT": xTs[b],
            "ctxT": ctxTs[b],
            "qw": np.ascontiguousarray(
                (np.asarray(q_w, np.float32)[:, el] * scale)).astype(np.float16),
            "kw": np.ascontiguousarray(
                np.asarray(k_w, np.float32)[:, el]).astype(np.float16),
            "vw": np.ascontiguousarray(
                np.asarray(v_w, np.float32)[:, el]).astype(np.float16),
            "ow": np.ascontiguousarray(
                np.asarray(o_w, np.float32)[el, :]).astype(np.float16),
            "kqb": np.ascontiguousarray(np.stack(
                [np.asarray(k_b, np.float32)[el],
                 np.asarray(q_b, np.float32)[el] * scale], axis=1)),
            "vbones": np.ascontiguousarray(np.concatenate(
                [np.asarray(v_b, np.float32)[el],
                 np.ones(128, np.float32)])[None, :]).astype(np.float16),
            "ident": np.eye(128, dtype=np.float16),
        })

    res = run_bass_kernel_spmd(nc, in_maps, list(range(N_CORES)))
    _last_results = res

    ob = np.asarray(o_b, np.float32)
    full = np.empty((B, S, E), np.float32)
    for b in range(B):
        full[b] = (res.results[2 * b]["out"].astype(np.float32)
                   + res.results[2 * b + 1]["out"].astype(np.float32) + ob)
    return full
